# revision 1
# baseline (speedup 1.0000x reference)
"""DEVISE margin hinge loss on 8 Trainium2 NeuronCores (Bass/Tile).

Data-parallel: batch sharded 8 ways, weights + label embeddings replicated.
Per core: proj = X_s @ W on PE; sims tiles = proj @ E.T on PE (row-tiled
K=64 pairs); hinge relu(margin + sims - true_sim) fused with the reduction
on ACT (activation+accum_out) and DVE (tensor_scalar+accum_out) reading
PSUM directly; partial scalar per core; host sums and normalizes.
"""

import numpy as np

B, D, C, DC = 4096, 1024, 20000, 64
MARGIN = 0.1
NCORES = 8
BL = B // NCORES           # 512 local batch
M_CHUNKS = BL // 128       # 4
K_CHUNKS = D // 128        # 8
CP_HALF = 10240            # packed half width (classes per 64-partition half)
N_PAD = 2 * CP_HALF - C    # 480 zero columns in the upper half
ET_TILES = 5               # (128, 2048) SBUF tiles covering ET_pack
G_PER_M = 10               # 1024-wide column groups per m-chunk sweep

_cache = {}


def _build_nc(reps: int = 1, variant: str = "full"):
    import concourse.bacc as bacc
    import concourse.mybir as mybir
    import concourse.tile as tile

    dt = mybir.dt.float32
    bf = mybir.dt.bfloat16
    Act = mybir.ActivationFunctionType
    Alu = mybir.AluOpType

    # Bacc (not raw Bass): its compile pass splits semaphore waits so no
    # engine instruction carries more than one (a TRN2 codegen constraint).
    nc = bacc.Bacc()
    xt_d = nc.declare_dram_parameter("xt", [128, K_CHUNKS * BL], bf, isOutput=False)
    w_d = nc.declare_dram_parameter("w", [128, K_CHUNKS * DC], bf, isOutput=False)
    et_d = nc.declare_dram_parameter("et", [128, CP_HALF], bf, isOutput=False)
    ey_d = nc.declare_dram_parameter("ey", [128, M_CHUNKS * DC], dt, isOutput=False)
    out_d = nc.declare_dram_parameter("out", [1, 1], dt, isOutput=True)

    # Per-chunk consumer costs (ns) used to load-balance ACT vs DVE.
    ACT_COST = (512 + 172) / 1.2
    DVE_COST = (512 + 120) / 0.96
    N_CHUNKS = CP_HALF // 512  # 20 column chunks of 512 per half

    with tile.TileContext(nc) as tc:
        def body(_iv=None):
            with tc.tile_pool(name="const", bufs=1) as cpool:
                # ---- loads (xt split per k-chunk so each matmul waits on
                # at most one DMA queue; walrus allows only one sync wait on
                # a self-loading fp32 matmul) -------------------------------
                xt_sb = cpool.tile([128, K_CHUNKS * BL], bf, tag="xt")
                for k in range(K_CHUNKS):
                    nc.sync.dma_start(
                        xt_sb[:, k * BL : (k + 1) * BL],
                        xt_d[:, k * BL : (k + 1) * BL],
                    )
                w_sb = cpool.tile([128, K_CHUNKS * DC], bf, tag="w")
                nc.sync.dma_start(w_sb[:], w_d[:])
                ey_sb = cpool.tile([128, M_CHUNKS * DC], dt, tag="ey")
                nc.sync.dma_start(ey_sb[:], ey_d[:])
                et_sb = []
                for i in range(ET_TILES):
                    t = cpool.tile([128, 2048], bf, tag=f"et{i}")
                    nc.sync.dma_start(t[:], et_d[:, i * 2048 : (i + 1) * 2048])
                    et_sb.append(t)

                projT_dup = cpool.tile([128, BL], bf, tag="projT")
                t_col = cpool.tile([128, M_CHUNKS], dt, tag="tcol")
                bias_col = cpool.tile([128, M_CHUNKS], dt, tag="bias")
                # separate per-engine accumulator tiles: a shared tile would
                # WAW-serialize the ACT and DVE consumer streams
                NTILES = M_CHUNKS * G_PER_M
                stats_a = cpool.tile([128, NTILES], dt, tag="stats_a")
                stats_d = cpool.tile([128, NTILES], dt, tag="stats_d")
                # unassigned columns must read as zero for the final reduce
                nc.gpsimd.memset(stats_a[:], 0.0)
                nc.vector.memset(stats_d[:], 0.0)
                zeros = cpool.tile([128, 1024], dt, tag="zeros")
                nc.vector.memset(zeros[:], 0.0)
                pscr = cpool.tile([128, DC], dt, tag="pscr")
                # single-buffer scratch, each written by exactly one engine:
                # WAW between consecutive consumers is same-engine program
                # order, so Tile emits no semaphores (and Bacc no event-sem
                # splits) in the hot loop
                a_scr = cpool.tile([128, 1024], dt, tag="ascr")
                d_scr = cpool.tile([128, 1024], dt, tag="dscr")

                if variant == "dma":
                    with tc.tile_pool(name="pdma", bufs=1, space="PSUM") as pd:
                        total_s = cpool.tile([1, 1], dt, tag="totscalar")
                        for t in [*et_sb, xt_sb, w_sb, ey_sb]:
                            tt = pd.tile([1, 1], dt, tag="touch")
                            nc.tensor.matmul(
                                tt[:], t[:, 0:1], t[:, 0:1], start=True, stop=True
                            )
                        nc.vector.memset(total_s[:], 0.0)
                        nc.sync.dma_start(out_d[:], total_s[:])
                    return

                # ---- phase 1: projT duplicated into both PSUM halves via
                # column tiling (partition-crossing copies are impossible on
                # ACT/DVE), then one aligned copy to SBUF ---------------------
                with (
                    tc.tile_pool(name="ppre", bufs=1, space="PSUM") as ppre,
                    tc.tile_pool(name="pbp", bufs=1, space="PSUM") as pbp,
                ):
                    def pe_touch(ap):
                        # 1x1 matmul reading `ap` only: absorbs one
                        # cross-engine wait so the next real matmul carries
                        # at most one.
                        tt = ppre.tile([1, 1], dt, tag="touch")
                        nc.tensor.matmul(
                            tt[:], ap[:, 0:1], ap[:, 0:1], start=True, stop=True
                        )

                    pe_touch(w_sb)  # absorb the w DMA wait
                    # two column-tiled chains into two different PSUM banks (a
                    # start=True clears has_written bits bank-wide, so
                    # interleaved groups must not share a bank)
                    psum_proj = ppre.tile([128, 2 * BL], dt, tag="pp")
                    for k in range(K_CHUNKS):
                        lhs = w_sb[:, k * DC : (k + 1) * DC]
                        rhs = xt_sb[:, k * BL : (k + 1) * BL]
                        nc.tensor.matmul(
                            psum_proj[0:64, 0:BL], lhs, rhs,
                            start=(k == 0), stop=(k == K_CHUNKS - 1),
                        )
                        nc.tensor.matmul(
                            psum_proj[64:128, BL : 2 * BL], lhs, rhs,
                            start=(k == 0), stop=(k == K_CHUNKS - 1),
                            tile_position=(0, 64),
                        )
                    nc.scalar.copy(projT_dup[0:64, :], psum_proj[0:64, 0:BL])
                    nc.scalar.copy(projT_dup[64:128, :], psum_proj[64:128, BL : 2 * BL])

                    # proj in batch-major layout, one m-chunk at a time, for t_b
                    for m in range(M_CHUNKS):
                        psum_pb = pbp.tile([128, DC], dt, tag="pb")
                        for k in range(K_CHUNKS):
                            nc.tensor.matmul(
                                psum_pb[:],
                                xt_sb[:, k * BL + m * 128 : k * BL + (m + 1) * 128],
                                w_sb[:, k * DC : (k + 1) * DC],
                                start=(k == 0),
                                stop=(k == K_CHUNKS - 1),
                            )
                        nc.vector.tensor_mul(
                            pscr[:], psum_pb[:], ey_sb[:, m * DC : (m + 1) * DC]
                        )
                        nc.vector.tensor_reduce(
                            t_col[:, m : m + 1], pscr[:],
                            axis=mybir.AxisListType.X, op=Alu.add,
                        )
                    pe_touch(projT_dup)

                # bias = margin - t
                nc.vector.tensor_scalar(
                    bias_col[:], t_col[:], -1.0, MARGIN, op0=Alu.mult, op1=Alu.add
                )

                # ---- phase 2: hinge sweep ----------------------------------
                # (128, 2048) PSUM slots: 4 matmuls fill 4 banks (lo classes
                # in banks 0-1, hi classes in banks 2-3); ACT consumes the lo
                # half while DVE consumes the hi half of the same slot -- they
                # touch different banks so they run in parallel, and the wide
                # ops amortize per-instruction overhead.
                it = 0
                with tc.tile_pool(name="ph2", bufs=2, space="PSUM") as p2:
                    for m in range(M_CHUNKS):
                        bias_m = bias_col[:, m : m + 1]
                        for g in range(G_PER_M):
                            eti, off = divmod(g, 2)
                            c0 = off * 1024
                            # separate lo/hi tiles: ACT and DVE never touch
                            # the same tile, so their reads can't serialize
                            slot_lo = p2.tile([128, 1024], dt, tag="plo")
                            slot_hi = p2.tile([128, 1024], dt, tag="phi")
                            if m == 0 and off == 0:
                                # absorb this et tile's DMA wait into a 1x1
                                # touch matmul (overwritten by the fill below)
                                nc.tensor.matmul(
                                    slot_lo[0:1, 0:1],
                                    et_sb[eti][:, 0:1],
                                    et_sb[eti][:, 0:1],
                                    start=True,
                                    stop=True,
                                )
                            for s in range(2):
                                cs = c0 + s * 512
                                nc.tensor.matmul(
                                    slot_lo[:, s * 512 : (s + 1) * 512],
                                    projT_dup[0:64, m * 128 : (m + 1) * 128],
                                    et_sb[eti][0:64, cs : cs + 512],
                                    start=True,
                                    stop=True,
                                    tile_position=(0, 0),
                                )
                                nc.tensor.matmul(
                                    slot_hi[:, s * 512 : (s + 1) * 512],
                                    projT_dup[64:128, m * 128 : (m + 1) * 128],
                                    et_sb[eti][64:128, cs : cs + 512],
                                    start=True,
                                    stop=True,
                                    tile_position=(64, 0),
                                )
                            if variant == "nocons":
                                it += 1
                                continue
                            nc.scalar.activation(
                                a_scr[:], slot_lo[:], Act.Relu,
                                bias=bias_m, scale=1.0,
                                accum_out=stats_a[:, it : it + 1],
                            )
                            nc.vector.scalar_tensor_tensor(
                                out=d_scr[:],
                                in0=slot_hi[:],
                                scalar=bias_m,
                                in1=zeros[:],
                                op0=Alu.add,
                                op1=Alu.max,
                                accum_out=stats_d[:, it : it + 1],
                            )
                            it += 1

                    # ---- phase 3: corrections + final scalar ----------------
                    padscr = cpool.tile([128, M_CHUNKS], dt, tag="padscr")
                    padsum = cpool.tile([128, 1], dt, tag="padsum")
                    # bias_col already holds margin - t; sum relu over m-chunks
                    nc.scalar.activation(
                        padscr[:],
                        bias_col[:],
                        Act.Relu,
                        bias=0.0,
                        scale=1.0,
                        accum_out=padsum[:],
                    )
                    red_a = cpool.tile([128, 1], dt, tag="red_a")
                    nc.vector.tensor_reduce(
                        red_a[:], stats_a[:], axis=mybir.AxisListType.X, op=Alu.add
                    )
                    red_d = cpool.tile([128, 1], dt, tag="red_d")
                    nc.vector.tensor_reduce(
                        red_d[:], stats_d[:], axis=mybir.AxisListType.X, op=Alu.add
                    )
                    tmp_col = cpool.tile([128, 1], dt, tag="tmp_col")
                    nc.vector.scalar_tensor_tensor(
                        out=tmp_col[:],
                        in0=padsum[:],
                        scalar=float(-N_PAD),
                        in1=red_a[:],
                        op0=Alu.mult,
                        op1=Alu.add,
                    )
                    total_col = cpool.tile([128, 1], dt, tag="total")
                    nc.vector.tensor_add(total_col[:], tmp_col[:], red_d[:])
                    ones_col = cpool.tile([128, 1], dt, tag="ones")
                    nc.vector.memset(ones_col[:], 1.0)
                    total_s = cpool.tile([1, 1], dt, tag="totscalar")
                    fin_slot = p2.tile([128, 1024], dt, tag="plo")
                    # touch absorbs the DVE wait for total_col, then the real
                    # 1x1 matmul sums total_col over partitions via ones
                    nc.tensor.matmul(
                        fin_slot[0:1, 0:1], total_col[:], total_col[:],
                        start=True, stop=True,
                    )
                    nc.tensor.matmul(
                        fin_slot[0:1, 0:1], total_col[:], ones_col[:],
                        start=True, stop=True,
                    )
                    nc.vector.tensor_copy(total_s[:], fin_slot[0:1, 0:1])
                nc.sync.dma_start(out_d[:], total_s[:])

        if reps == 1:
            body()
        else:
            with tc.For_i(0, reps, 1) as iv:
                body(iv)

    nc.finalize()
    return nc


def _pack_inputs(X, y, E, W):
    """Per-core DRAM images. Layouts match the device program above."""
    import ml_dtypes

    bf16 = ml_dtypes.bfloat16
    X = np.ascontiguousarray(np.asarray(X, dtype=np.float32))
    y = np.asarray(y).astype(np.int64)
    E = np.ascontiguousarray(np.asarray(E, dtype=np.float32))
    W = np.ascontiguousarray(np.asarray(W, dtype=np.float32))

    w_pack = np.ascontiguousarray(
        W.reshape(K_CHUNKS, 128, DC).transpose(1, 0, 2).reshape(128, K_CHUNKS * DC)
    ).astype(bf16)
    Et = E.T  # (64, C)
    et_pack = np.zeros((128, CP_HALF), dtype=np.float32)
    et_pack[:64, :] = Et[:, :CP_HALF]
    et_pack[64:, : C - CP_HALF] = Et[:, CP_HALF:]
    et_pack = np.ascontiguousarray(et_pack.astype(bf16))

    in_maps = []
    for s in range(NCORES):
        Xs = X[s * BL : (s + 1) * BL]  # (BL, D)
        xt_pack = np.ascontiguousarray(
            Xs.T.reshape(K_CHUNKS, 128, BL).transpose(1, 0, 2).reshape(128, K_CHUNKS * BL)
        ).astype(bf16)
        Ey = E[y[s * BL : (s + 1) * BL]]  # (BL, DC)
        ey_pack = np.ascontiguousarray(
            Ey.reshape(M_CHUNKS, 128, DC).transpose(1, 0, 2).reshape(128, M_CHUNKS * DC)
        )
        in_maps.append({"xt": xt_pack, "w": w_pack, "et": et_pack, "ey": ey_pack})
    return in_maps


def run_spmd(in_maps, reps: int = 1, trace: bool = False):
    from concourse.bass_utils import run_bass_kernel_spmd

    key = reps
    if key not in _cache:
        _cache[key] = _build_nc(reps)  # full variant only
    nc = _cache[key]
    return run_bass_kernel_spmd(
        nc, in_maps, core_ids=list(range(NCORES)), trace=trace
    )


def kernel(X, y, label_embeddings, weights):
    in_maps = _pack_inputs(X, y, label_embeddings, weights)
    res = run_spmd(in_maps).results
    total = sum(float(res[s]["out"][0, 0]) for s in range(NCORES))
    loss = np.float32(total / B - MARGIN)
    return np.array([loss], dtype=np.float32)

